# revision 21
# baseline (speedup 1.0000x reference)
"""Trainium2 Bass kernel for nn_DCELoss (decoupled contrastive-style loss).

The whole loss reduces to three 32x32 gram matrices over the flattened
feature axis K = 96^3 = 884736:
    G_pp = p @ p.T,  G_ph = p @ h.T,  G_hh = h @ h.T
(row norms are their diagonals).  The final masked reduction is tiny 32x32
math done on host in float64.

Sharding: data-parallel over K across the 8 NeuronCores.  Each core gets a
K/8 slice, pre-packed on host into a transposed + interleaved fp8 layout
X[128, 432, 2, 64]: group g holds two 128-k chunks, each as 64 columns
[p_rows(32) | h_rows(32)].  On device each group is fed to the PE as BOTH
stationary and moving operand of a DoubleRow fp8 matmul:
    psum[64,64] += X[:,g,0,:].T @ X[:,g,0,:] + X[:,g,1,:].T @ X[:,g,1,:]
i.e. the exact 64x64 gram over 256 k-values per instruction, with no
wasted off-diagonal compute (the old S^T S formulation burned half the PE
throughput on unused cross-chunk blocks).  DoubleRow runs fp8 at 0.5
cycles/row, so the PE consumes a 16 KiB group in ~27-40 ns while the DMA
delivers one every ~75 ns: the kernel is DMA-bound at the ~435 GB/s
per-core HBM-read rate, and the HAM utilization ramp is absorbed while the
first segments stream in (no dummy-matmul warmup needed).

fp8_e4m3 quantization of the inputs perturbs the final loss by ~3e-6
relative: the loss is a log of large masked sums of exp(cosine) terms with
cosines ~1e-3 over K ~ 1e6 elements, so elementwise rounding noise cancels
almost entirely.

Raw Bass (no Tile framework): the dependency structure is a static
producer-consumer chain, and skipping Tile's all-engine preamble barrier +
kernel-tail EVSEM butterfly saves >10us on a ~25us kernel.
"""

import os
import numpy as np

B = 32
K = 884736
NCORES = 8
KC = K // NCORES            # 110592 k-values per core
NCH = KC // 128             # 864 chunks of 128 k-values
GROUPS = NCH // 2           # 432 DoubleRow matmul groups (2 chunks each)
FREE = NCH * 2 * B          # 55296 free columns of X per core
# Input DMA segments, in units of groups (16 KiB each; total 432).  A small
# first segment lets the PE start early; large middle segments give 4-8 KiB
# per-partition DMA lines (full SDMA rate); small tail segments so the last
# matmuls finish right behind the last DMA byte.  Segments alternate between
# the HWDGE rings listed in RING_ENGINES.
SEG_GROUPS = [8, 16, 32, 56, 64, 64, 64, 56, 40, 20, 12]
RING_ENGINES = ("sync", "scalar")
assert sum(SEG_GROUPS) == GROUPS
NSEG = len(SEG_GROUPS)
# Dense N=128 dummy matmuls before the data phase: the PE HAM clock-gate
# promotes 4/8 -> 8/8 only after ~3.4-6 us of near-100% PE duty; DoubleRow
# data matmuls alone (~70% duty at the cold clock) never trigger promotion
# and the whole phase runs at 1.2 GHz (measured: 73 ns/group vs ~30 warm).
WARMUP_MMS = 55

_CACHE = {}
LAST_RESULT = None  # BassKernelResults of the most recent run (for test harness)


def _f8_dtype():
    import ml_dtypes

    return ml_dtypes.float8_e4m3


def _ensure_ntff_hook():
    """Install antenv.axon_hooks shim if missing, so run_bass_kernel_spmd
    trace=True can capture NTFF profiles via libaxon_pjrt.so ctypes calls.
    Only used when tracing is requested (test harness)."""
    import sys
    try:
        from antenv.axon_hooks import get_axon_ntff_profile_hook  # noqa: F401
        return
    except ImportError:
        pass
    import ctypes
    import contextlib
    import types

    so_path = "/opt/axon/libaxon_pjrt.so"
    hook = None
    if os.path.exists(so_path):
        lib = ctypes.CDLL(so_path)
        if hasattr(lib, "axon_start_nrt_profile"):
            lib.axon_start_nrt_profile.argtypes = [
                ctypes.POINTER(ctypes.c_int64),
                ctypes.c_size_t,
            ]
            lib.axon_start_nrt_profile.restype = ctypes.c_int64
            lib.axon_stop_nrt_profile.argtypes = [ctypes.c_char_p]
            lib.axon_stop_nrt_profile.restype = ctypes.c_int64

            @contextlib.contextmanager
            def _hook(output_dir, device_ids):
                import jax

                jax.devices()
                if device_ids:
                    ids = (ctypes.c_int64 * len(device_ids))(*device_ids)
                    rc = lib.axon_start_nrt_profile(ids, len(device_ids))
                else:
                    rc = lib.axon_start_nrt_profile(None, 0)
                if rc != 0:
                    raise RuntimeError(f"axon_start_nrt_profile rc={rc}")
                try:
                    yield
                finally:
                    n = lib.axon_stop_nrt_profile(str(output_dir).encode())
                    if n < 0:
                        raise RuntimeError(f"axon_stop_nrt_profile rc={n}")
                    print(f"profile: {n} file(s) written to {output_dir}")

            hook = _hook

    mod = types.ModuleType("antenv.axon_hooks")
    mod._hook = hook
    mod.get_axon_ntff_profile_hook = lambda: mod._hook
    mod.set_axon_ntff_profile_hook = lambda h: setattr(mod, "_hook", h)
    import antenv

    antenv.axon_hooks = mod
    sys.modules["antenv.axon_hooks"] = mod


def _build():
    """Build the per-core Bass program (SPMD, identical on all cores).

    Raw Bass with manual semaphores:
      sync/scalar : input dma_starts (queued back-to-back, one ring each),
                    sync also stores the PSUM gram to DRAM at the end
      tensor      : per segment wait for its DMA, then run its DoubleRow
                    LDW+MM pairs, all accumulating into one PSUM bank
    """
    import concourse.bass as bass
    import concourse.mybir as mybir

    nc = bass.Bass(
        "TRN2",
        target_bir_lowering=False,
        debug=False,
        enable_asserts=False,
        num_devices=NCORES,
        enable_partition_id=False,
    )
    x = nc.dram_tensor(
        "x", [128, GROUPS, 2, 64], mybir.dt.float8e4, kind="ExternalInput"
    )
    out = nc.dram_tensor("out", [128, 192], mybir.dt.float32, kind="ExternalOutput")

    import contextlib

    with contextlib.ExitStack() as ctx:
        xsb = ctx.enter_context(
            nc.sbuf_tensor([128, GROUPS, 2, 64], mybir.dt.float8e4)
        )
        osb = ctx.enter_context(nc.sbuf_tensor([128, 192], mybir.dt.float32))
        wsb = ctx.enter_context(nc.sbuf_tensor([128, 128], mybir.dt.float8e4))
        # Two full PSUM banks.  Bank 0 (free cols 0:512) holds the plain
        # S^T S accumulator [128,128] whose [0:64,0:64] quadrant doubles as
        # the tiled even-chunk gram; bank 1 (free cols 512:1024) holds the
        # tiled odd-chunk gram at partitions 64-127.  Disjoint zero regions
        # AND disjoint PE column groups -> the two tiled matmuls of a group
        # stream concurrently through separate XBUSes.
        ps = ctx.enter_context(nc.psum_tensor([128, 1024], mybir.dt.float32))
        wps = ctx.enter_context(nc.psum_tensor([128, 128], mybir.dt.float32))
        seg_sems = [
            ctx.enter_context(nc.semaphore(name=f"seg_sem{s}")) for s in range(NSEG)
        ]
        warm_sem = ctx.enter_context(nc.semaphore(name="warm_sem"))
        mm_done = ctx.enter_context(nc.semaphore(name="mm_done"))
        copy_done = ctx.enter_context(nc.semaphore(name="copy_done"))
        out_sem = ctx.enter_context(nc.semaphore(name="out_sem"))
        block = ctx.enter_context(nc.Block())

        seg_start = [sum(SEG_GROUPS[:s]) for s in range(NSEG)]

        def issue_loads(eng, segs):
            for s in segs:
                g0, gn = seg_start[s], SEG_GROUPS[s]
                eng.dma_start(
                    out=xsb[:, g0 : g0 + gn], in_=x[:, g0 : g0 + gn]
                ).then_inc(seg_sems[s], 16)

        ring_segs = {
            e: [s for s in range(NSEG) if RING_ENGINES[s % len(RING_ENGINES)] == e]
            for e in RING_ENGINES
        }

        @block.sync
        def _(sync):
            issue_loads(sync, ring_segs.get("sync", []))
            sync.wait_ge(copy_done, 1)
            sync.dma_start(out=out[:], in_=osb[:]).then_inc(out_sem, 16)
            sync.wait_ge(out_sem, 16)

        @block.scalar
        def _(scalar):
            issue_loads(scalar, ring_segs.get("scalar", []))

        @block.vector
        def _(vector):
            vector.memset(osb[0:64, 128:192], 0.0)
            vector.wait_ge(mm_done, 1)
            vector.tensor_copy(osb[:, 0:128], ps[:, 0:128])
            vector.tensor_copy(osb[64:128, 128:192], ps[64:128, 512:576]).then_inc(
                copy_done, 1
            )

        @block.gpsimd
        def _(gpsimd):
            gpsimd.memset(wsb[:], 0.0).then_inc(warm_sem, 1)

        @block.tensor
        def _(tensor):
            tensor.wait_ge(warm_sem, 1)
            for _ in range(WARMUP_MMS):
                tensor.matmul(wps[:], wsb[:], wsb[:], start=True, stop=True)
            # Mix: every 3rd group (g % 3 == 0) runs as one full-width plain
            # S^T S matmul (N=128, whole array busy 53 ns); the rest as two
            # concurrent col-tiled N=64 gram matmuls (~30 ns).  Average PE
            # rate ~38.7 ns/group ~= the 37.6 ns/group DMA delivery rate at
            # ~92% array duty, which holds the HAM clock-gate at 8/8 (pure
            # tiling idles ~25% and HAM demotes to 4/8 mid-phase; measured).
            # Group 0 is plain so the first write covers PSUM bank 0's whole
            # zero region uniformly.
            g = 0
            for s in range(NSEG):
                tensor.wait_ge(seg_sems[s], 16)
                for j in range(SEG_GROUPS[s]):
                    if g % 3 == 0 or g == GROUPS - 1:
                        sl = xsb[:, seg_start[s] + j]
                        mm = tensor.matmul(
                            ps[:, 0:128], sl, sl,
                            start=(g == 0), stop=(g == GROUPS - 1),
                        )
                    else:
                        te = xsb[:, seg_start[s] + j, 0]
                        to = xsb[:, seg_start[s] + j, 1]
                        tensor.matmul(
                            ps[0:64, 0:64], te, te,
                            start=False, stop=False,
                        )
                        mm = tensor.matmul(
                            ps[64:128, 512:576], to, to,
                            start=(g == 1), stop=(g == GROUPS - 2),
                        )
                    g += 1
            mm.then_inc(mm_done, 1)

    return nc


def _prepare_inputs(pred, hr):
    """Pack p/h into the per-core transposed+interleaved fp8 layout.

    X[core][q, c, t, j] = (p if t==0 else h)[j, core*KC + c*128 + q]
    flattened to [128, GROUPS, 2, 64] per core (c = 2*g + parity, with the
    two chunks of group g side by side in the last-two axes as
    [p|h]_even, [p|h]_odd -> [128, g, (even|odd), (p32|h32)]).
    """
    f8 = _f8_dtype()
    p = np.asarray(pred).reshape(B, K).astype(f8)
    h = np.asarray(hr).reshape(B, K).astype(f8)
    p4 = p.reshape(B, NCORES, NCH, 128)
    h4 = h.reshape(B, NCORES, NCH, 128)
    xall = np.empty((NCORES, 128, NCH, 2, B), dtype=f8)
    xall[:, :, :, 0, :] = p4.transpose(1, 3, 2, 0)
    xall[:, :, :, 1, :] = h4.transpose(1, 3, 2, 0)
    return xall.reshape(NCORES, 128, GROUPS, 2, 64)


def _finalize(R):
    """R: [128,192] float64 sum of per-core accumulated PSUM banks.
    Cols 0:128 = bank 0: the plain S^T S accumulator whose [0:64,0:64]
    quadrant also holds the tiled even-chunk grams and whose [64:128,64:128]
    quadrant holds the plain groups' odd-chunk grams (off-diagonal quadrants
    are unused cross-chunk garbage).  Cols 128:192 at partitions 64:128 =
    bank 1: the tiled odd-chunk grams.  Within a 64x64 gram block,
    rows/cols 0..31 = pred rows, 32..63 = hr rows."""
    R = R[0:64, 0:64] + R[64:128, 64:128] + R[64:128, 128:192]
    Gpp = R[0:32, 0:32]
    Gph = R[0:32, 32:64]
    Ghh = R[32:64, 32:64]

    pn = np.sqrt(np.diag(Gpp))
    hn = np.sqrt(np.diag(Ghh))
    S_srhr = Gph / (pn[:, None] * hn[None, :])
    S_srsr = Gpp / (pn[:, None] * pn[None, :])
    hsq = np.diag(Ghh)
    d2 = np.maximum(hsq[:, None] + hsq[None, :] - 2.0 * Ghh, 0.0)
    dist = np.sqrt(d2)
    with np.errstate(divide="ignore"):
        M = np.minimum(-20.0 * np.log10(dist), 0.0)
    mask_pos = np.abs(M) > 30.0
    w = (np.exp(S_srsr) + 2.0 * np.exp(S_srhr)) / 0.5
    Qpos = np.where(mask_pos, w, 0.0).sum(axis=1)
    Qneg = np.where(mask_pos, 0.0, w).sum(axis=1)
    loss = (-1.0 / B) * np.sum(np.log(Qpos / Qneg))
    return np.asarray(loss, dtype=np.float32)


def kernel(pred, hr):
    global LAST_RESULT
    from concourse.bass_utils import run_bass_kernel_spmd

    trace = bool(os.environ.get("KERNEL_TRACE"))
    if trace:
        _ensure_ntff_hook()

    if "nc" not in _CACHE:
        _CACHE["nc"] = _build()
    nc = _CACHE["nc"]

    xall = _prepare_inputs(pred, hr)
    in_maps = [{"x": xall[c]} for c in range(NCORES)]
    # The axon-tunneled NeuronCores occasionally report a transient
    # unrecoverable-exec-unit error; recovery can take tens of seconds,
    # so back off with escalating sleeps before resubmitting.
    last_err = None
    res = None
    for attempt, backoff in enumerate([10.0, 30.0, 90.0, 0.0]):
        try:
            res = run_bass_kernel_spmd(
                nc, in_maps, core_ids=list(range(NCORES)), trace=trace and attempt == 0
            )
            break
        except Exception as e:  # noqa: BLE001
            last_err = e
            if backoff == 0.0:
                raise
            import time

            time.sleep(backoff)
    if res is None:
        raise last_err
    LAST_RESULT = res
    R = np.zeros((128, 192), dtype=np.float64)
    for c in range(NCORES):
        R += res.results[c]["out"].astype(np.float64)
    return _finalize(R)


# revision 24
# speedup vs baseline: 1.6537x; 1.6537x over previous
"""Trainium2 Bass kernel for nn_DCELoss (decoupled contrastive-style loss).

The whole loss reduces to three 32x32 gram matrices over the flattened
feature axis K = 96^3 = 884736:
    G_pp = p @ p.T,  G_ph = p @ h.T,  G_hh = h @ h.T
(row norms are their diagonals).  The final masked reduction is tiny 32x32
math done on host in float64.

Sharding: data-parallel over K across the 8 NeuronCores.  Each core gets a
K/8 slice, pre-packed on host into a transposed + interleaved fp8 layout
X[128, 432, 2, 64]: group g holds two 128-k chunks, each as 64 columns
[p_rows(32) | h_rows(32)].  On device each group is fed to the PE as BOTH
stationary and moving operand of a DoubleRow fp8 matmul:
    psum[64,64] += X[:,g,0,:].T @ X[:,g,0,:] + X[:,g,1,:].T @ X[:,g,1,:]
i.e. the exact 64x64 gram over 256 k-values per instruction, with no
wasted off-diagonal compute (the old S^T S formulation burned half the PE
throughput on unused cross-chunk blocks).  DoubleRow runs fp8 at 0.5
cycles/row, so the PE consumes a 16 KiB group in ~27-40 ns while the DMA
delivers one every ~75 ns: the kernel is DMA-bound at the ~435 GB/s
per-core HBM-read rate, and the HAM utilization ramp is absorbed while the
first segments stream in (no dummy-matmul warmup needed).

fp8_e4m3 quantization of the inputs perturbs the final loss by ~3e-6
relative: the loss is a log of large masked sums of exp(cosine) terms with
cosines ~1e-3 over K ~ 1e6 elements, so elementwise rounding noise cancels
almost entirely.

Raw Bass (no Tile framework): the dependency structure is a static
producer-consumer chain, and skipping Tile's all-engine preamble barrier +
kernel-tail EVSEM butterfly saves >10us on a ~25us kernel.
"""

import os
import numpy as np

B = 32
K = 884736
NCORES = 8
KC = K // NCORES            # 110592 k-values per core
NCH = KC // 128             # 864 chunks of 128 k-values
GROUPS = NCH // 2           # 432 col-tiled matmul groups (2 chunks each)
FREE = NCH * 2 * B          # 55296 free columns of X per core
# Input DMA segments, in units of groups (16 KiB each; total 432).  A small
# first segment lets the PE start early; large middle segments give 4-8 KiB
# per-partition DMA lines (full SDMA rate); small tail segments so the last
# matmuls finish right behind the last DMA byte.  Segments alternate between
# the HWDGE rings listed in RING_ENGINES.
SEG_GROUPS = [8, 16, 32, 56, 64, 64, 64, 56, 40, 20, 12]
RING_ENGINES = ("sync", "scalar")
assert sum(SEG_GROUPS) == GROUPS
NSEG = len(SEG_GROUPS)
# Dense N=128 dummy matmuls before the data phase: the PE HAM clock-gate
# promotes 4/8 -> 8/8 only after ~5-6 us of near-100% PE duty; col-tiled
# data matmuls alone (~70% duty at the cold clock) never trigger promotion
# and the whole phase runs at 1.2 GHz (measured: 73 ns/group vs ~30 warm).
WARMUP_MMS = 55
# Moving-stream length per tiled matmul.  64 is the useful width; the extra
# columns are deliberate idle-padding for the HAM clock-gate: at N=64 the
# array is only ~70% busy at the 37.6 ns/group DMA pace and HAM demotes to
# 4/8 mid-phase (measured, costs ~2x).  N=80 keeps ~89% duty at a PE rate
# that just matches the DMA delivery rate.
NMOV = 80

_CACHE = {}
LAST_RESULT = None  # BassKernelResults of the most recent run (for test harness)


def _f8_dtype():
    import ml_dtypes

    return ml_dtypes.float8_e4m3


def _ensure_ntff_hook():
    """Install antenv.axon_hooks shim if missing, so run_bass_kernel_spmd
    trace=True can capture NTFF profiles via libaxon_pjrt.so ctypes calls.
    Only used when tracing is requested (test harness)."""
    import sys
    try:
        from antenv.axon_hooks import get_axon_ntff_profile_hook  # noqa: F401
        return
    except ImportError:
        pass
    import ctypes
    import contextlib
    import types

    so_path = "/opt/axon/libaxon_pjrt.so"
    hook = None
    if os.path.exists(so_path):
        lib = ctypes.CDLL(so_path)
        if hasattr(lib, "axon_start_nrt_profile"):
            lib.axon_start_nrt_profile.argtypes = [
                ctypes.POINTER(ctypes.c_int64),
                ctypes.c_size_t,
            ]
            lib.axon_start_nrt_profile.restype = ctypes.c_int64
            lib.axon_stop_nrt_profile.argtypes = [ctypes.c_char_p]
            lib.axon_stop_nrt_profile.restype = ctypes.c_int64

            @contextlib.contextmanager
            def _hook(output_dir, device_ids):
                import jax

                jax.devices()
                if device_ids:
                    ids = (ctypes.c_int64 * len(device_ids))(*device_ids)
                    rc = lib.axon_start_nrt_profile(ids, len(device_ids))
                else:
                    rc = lib.axon_start_nrt_profile(None, 0)
                if rc != 0:
                    raise RuntimeError(f"axon_start_nrt_profile rc={rc}")
                try:
                    yield
                finally:
                    n = lib.axon_stop_nrt_profile(str(output_dir).encode())
                    if n < 0:
                        raise RuntimeError(f"axon_stop_nrt_profile rc={n}")
                    print(f"profile: {n} file(s) written to {output_dir}")

            hook = _hook

    mod = types.ModuleType("antenv.axon_hooks")
    mod._hook = hook
    mod.get_axon_ntff_profile_hook = lambda: mod._hook
    mod.set_axon_ntff_profile_hook = lambda h: setattr(mod, "_hook", h)
    import antenv

    antenv.axon_hooks = mod
    sys.modules["antenv.axon_hooks"] = mod


def _build():
    """Build the per-core Bass program (SPMD, identical on all cores).

    Raw Bass with manual semaphores:
      sync/scalar : input dma_starts (queued back-to-back, one ring each),
                    sync also stores the PSUM gram to DRAM at the end
      tensor      : per segment wait for its DMA, then run its DoubleRow
                    LDW+MM pairs, all accumulating into one PSUM bank
    """
    import concourse.bass as bass
    import concourse.mybir as mybir

    nc = bass.Bass(
        "TRN2",
        target_bir_lowering=False,
        debug=False,
        enable_asserts=False,
        num_devices=NCORES,
        enable_partition_id=False,
    )
    x = nc.dram_tensor(
        "x", [128, GROUPS, 128], mybir.dt.float8e4, kind="ExternalInput"
    )
    out = nc.dram_tensor("out", [128, 64], mybir.dt.float32, kind="ExternalOutput")

    import contextlib

    with contextlib.ExitStack() as ctx:
        xsb = ctx.enter_context(
            nc.sbuf_tensor([128, GROUPS, 128], mybir.dt.float8e4)
        )
        osb = ctx.enter_context(nc.sbuf_tensor([128, 64], mybir.dt.float32))
        wsb = ctx.enter_context(nc.sbuf_tensor([128, 128], mybir.dt.float8e4))
        # Two full PSUM banks.  Bank 0 (free cols 0:512) holds the
        # even-chunk gram accumulator at partitions 0-63 (its cols 64:80 are
        # the NMOV cross-chunk garbage tail); bank 1 (free cols 512:1024)
        # holds the odd-chunk gram at partitions 64-127 (useful block at the
        # END of its 80-col window).  Disjoint zero regions AND disjoint PE
        # column groups -> the two matmuls of a group stream concurrently
        # through separate XBUSes.
        ps = ctx.enter_context(nc.psum_tensor([128, 1024], mybir.dt.float32))
        wps = ctx.enter_context(nc.psum_tensor([128, 128], mybir.dt.float32))
        seg_sems = [
            ctx.enter_context(nc.semaphore(name=f"seg_sem{s}")) for s in range(NSEG)
        ]
        warm_sem = ctx.enter_context(nc.semaphore(name="warm_sem"))
        mm_done = ctx.enter_context(nc.semaphore(name="mm_done"))
        copy_done = ctx.enter_context(nc.semaphore(name="copy_done"))
        out_sem = ctx.enter_context(nc.semaphore(name="out_sem"))
        block = ctx.enter_context(nc.Block())

        seg_start = [sum(SEG_GROUPS[:s]) for s in range(NSEG)]

        def issue_loads(eng, segs):
            for s in segs:
                g0, gn = seg_start[s], SEG_GROUPS[s]
                eng.dma_start(
                    out=xsb[:, g0 : g0 + gn], in_=x[:, g0 : g0 + gn]
                ).then_inc(seg_sems[s], 16)

        ring_segs = {
            e: [s for s in range(NSEG) if RING_ENGINES[s % len(RING_ENGINES)] == e]
            for e in RING_ENGINES
        }

        @block.sync
        def _(sync):
            issue_loads(sync, ring_segs.get("sync", []))
            sync.wait_ge(copy_done, 1)
            sync.dma_start(out=out[:], in_=osb[:]).then_inc(out_sem, 16)
            sync.wait_ge(out_sem, 16)

        @block.scalar
        def _(scalar):
            issue_loads(scalar, ring_segs.get("scalar", []))

        @block.vector
        def _(vector):
            vector.wait_ge(mm_done, 1)
            vector.tensor_copy(osb[0:64, :], ps[0:64, 0:64])
            vector.tensor_copy(
                osb[64:128, :], ps[64:128, 512 + NMOV - 64 : 512 + NMOV]
            ).then_inc(copy_done, 1)

        @block.gpsimd
        def _(gpsimd):
            gpsimd.memset(wsb[:], 0.0).then_inc(warm_sem, 1)

        @block.tensor
        def _(tensor):
            tensor.wait_ge(warm_sem, 1)
            for _ in range(WARMUP_MMS):
                tensor.matmul(wps[:], wsb[:], wsb[:], start=True, stop=True)
            # Every group: two concurrent col-tiled gram matmuls.  The
            # even chunk's stationary occupies PE columns 0-63 (psum
            # partitions 0-63), the odd chunk's PE columns 64-127 (psum
            # partitions 64-127), each streaming an NMOV-column window of
            # the group's 128 columns: [0:NMOV] for even (gram in out cols
            # 0:64), [128-NMOV:128] for odd (gram in the last 64 out cols).
            g = 0
            for s in range(NSEG):
                tensor.wait_ge(seg_sems[s], 16)
                for j in range(SEG_GROUPS[s]):
                    gg = seg_start[s] + j
                    te = xsb[:, gg, 0:64]
                    to = xsb[:, gg, 64:128]
                    me = xsb[:, gg, 0:NMOV]
                    mo = xsb[:, gg, 128 - NMOV : 128]
                    tensor.matmul(
                        ps[0:64, 0:NMOV], te, me,
                        start=(g == 0), stop=(g == GROUPS - 1),
                    )
                    mm = tensor.matmul(
                        ps[64:128, 512 : 512 + NMOV], to, mo,
                        start=(g == 0), stop=(g == GROUPS - 1),
                    )
                    g += 1
            mm.then_inc(mm_done, 1)

    return nc


def _prepare_inputs(pred, hr):
    """Pack p/h into the per-core transposed+interleaved fp8 layout.

    X[core][q, c, t, j] = (p if t==0 else h)[j, core*KC + c*128 + q]
    flattened to [128, GROUPS, 2, 64] per core (c = 2*g + parity, with the
    two chunks of group g side by side in the last-two axes as
    [p|h]_even, [p|h]_odd -> [128, g, (even|odd), (p32|h32)]).
    """
    f8 = _f8_dtype()
    p = np.asarray(pred).reshape(B, K).astype(f8)
    h = np.asarray(hr).reshape(B, K).astype(f8)
    p4 = p.reshape(B, NCORES, NCH, 128)
    h4 = h.reshape(B, NCORES, NCH, 128)
    xall = np.empty((NCORES, 128, NCH, 2, B), dtype=f8)
    xall[:, :, :, 0, :] = p4.transpose(1, 3, 2, 0)
    xall[:, :, :, 1, :] = h4.transpose(1, 3, 2, 0)
    return xall.reshape(NCORES, 128, GROUPS, 128)


def _finalize(R):
    """R: [128,64] float64 sum of per-core accumulated gram matrices:
    partitions 0..63 hold the even-chunk gram, 64..127 the odd-chunk gram
    (the two col-tiled PE halves).  Rows/cols 0..31 = pred, 32..63 = hr."""
    R = R[0:64] + R[64:128]
    Gpp = R[0:32, 0:32]
    Gph = R[0:32, 32:64]
    Ghh = R[32:64, 32:64]

    pn = np.sqrt(np.diag(Gpp))
    hn = np.sqrt(np.diag(Ghh))
    S_srhr = Gph / (pn[:, None] * hn[None, :])
    S_srsr = Gpp / (pn[:, None] * pn[None, :])
    hsq = np.diag(Ghh)
    d2 = np.maximum(hsq[:, None] + hsq[None, :] - 2.0 * Ghh, 0.0)
    dist = np.sqrt(d2)
    with np.errstate(divide="ignore"):
        M = np.minimum(-20.0 * np.log10(dist), 0.0)
    mask_pos = np.abs(M) > 30.0
    w = (np.exp(S_srsr) + 2.0 * np.exp(S_srhr)) / 0.5
    Qpos = np.where(mask_pos, w, 0.0).sum(axis=1)
    Qneg = np.where(mask_pos, 0.0, w).sum(axis=1)
    loss = (-1.0 / B) * np.sum(np.log(Qpos / Qneg))
    return np.asarray(loss, dtype=np.float32)


def kernel(pred, hr):
    global LAST_RESULT
    from concourse.bass_utils import run_bass_kernel_spmd

    trace = bool(os.environ.get("KERNEL_TRACE"))
    if trace:
        _ensure_ntff_hook()

    if "nc" not in _CACHE:
        _CACHE["nc"] = _build()
    nc = _CACHE["nc"]

    xall = _prepare_inputs(pred, hr)
    in_maps = [{"x": xall[c]} for c in range(NCORES)]
    # The axon-tunneled NeuronCores occasionally report a transient
    # unrecoverable-exec-unit error; recovery can take tens of seconds,
    # so back off with escalating sleeps before resubmitting.
    last_err = None
    res = None
    for attempt, backoff in enumerate([10.0, 30.0, 90.0, 0.0]):
        try:
            res = run_bass_kernel_spmd(
                nc, in_maps, core_ids=list(range(NCORES)), trace=trace and attempt == 0
            )
            break
        except Exception as e:  # noqa: BLE001
            last_err = e
            if backoff == 0.0:
                raise
            import time

            time.sleep(backoff)
    if res is None:
        raise last_err
    LAST_RESULT = res
    R = np.zeros((128, 64), dtype=np.float64)
    for c in range(NCORES):
        R += res.results[c]["out"].astype(np.float64)
    return _finalize(R)


# revision 25
# speedup vs baseline: 1.6615x; 1.0047x over previous
"""Trainium2 Bass kernel for nn_DCELoss (decoupled contrastive-style loss).

The whole loss reduces to three 32x32 gram matrices over the flattened
feature axis K = 96^3 = 884736:
    G_pp = p @ p.T,  G_ph = p @ h.T,  G_hh = h @ h.T
(row norms are their diagonals).  The final masked reduction is tiny 32x32
math done on host in float64.

Sharding: data-parallel over K across the 8 NeuronCores.  Each core gets a
K/8 slice, pre-packed on host into a transposed + interleaved fp8 layout
X[128, 432, 2, 64]: group g holds two 128-k chunks, each as 64 columns
[p_rows(32) | h_rows(32)].  On device each group is fed to the PE as BOTH
stationary and moving operand of a DoubleRow fp8 matmul:
    psum[64,64] += X[:,g,0,:].T @ X[:,g,0,:] + X[:,g,1,:].T @ X[:,g,1,:]
i.e. the exact 64x64 gram over 256 k-values per instruction, with no
wasted off-diagonal compute (the old S^T S formulation burned half the PE
throughput on unused cross-chunk blocks).  DoubleRow runs fp8 at 0.5
cycles/row, so the PE consumes a 16 KiB group in ~27-40 ns while the DMA
delivers one every ~75 ns: the kernel is DMA-bound at the ~435 GB/s
per-core HBM-read rate, and the HAM utilization ramp is absorbed while the
first segments stream in (no dummy-matmul warmup needed).

fp8_e4m3 quantization of the inputs perturbs the final loss by ~3e-6
relative: the loss is a log of large masked sums of exp(cosine) terms with
cosines ~1e-3 over K ~ 1e6 elements, so elementwise rounding noise cancels
almost entirely.

Raw Bass (no Tile framework): the dependency structure is a static
producer-consumer chain, and skipping Tile's all-engine preamble barrier +
kernel-tail EVSEM butterfly saves >10us on a ~25us kernel.
"""

import os
import numpy as np

B = 32
K = 884736
NCORES = 8
KC = K // NCORES            # 110592 k-values per core
NCH = KC // 128             # 864 chunks of 128 k-values
GROUPS = NCH // 2           # 432 col-tiled matmul groups (2 chunks each)
FREE = NCH * 2 * B          # 55296 free columns of X per core
# Input DMA segments, in units of groups (16 KiB each; total 432).  A small
# first segment lets the PE start early; large middle segments give 4-8 KiB
# per-partition DMA lines (full SDMA rate); small tail segments so the last
# matmuls finish right behind the last DMA byte.  Segments alternate between
# the HWDGE rings listed in RING_ENGINES.
SEG_GROUPS = [8, 16, 32, 56, 64, 64, 64, 56, 40, 20, 12]
RING_ENGINES = ("sync", "scalar")
assert sum(SEG_GROUPS) == GROUPS
NSEG = len(SEG_GROUPS)
# Dense N=128 dummy matmuls before the data phase: the PE HAM clock-gate
# promotes 4/8 -> 8/8 only after ~5-6 us of near-100% PE duty; col-tiled
# data matmuls alone (~70% duty at the cold clock) never trigger promotion
# and the whole phase runs at 1.2 GHz (measured: 73 ns/group vs ~30 warm).
WARMUP_MMS = 55
# Moving-stream length per tiled matmul.  64 is the useful width; the extra
# columns are deliberate idle-padding for the HAM clock-gate: at N=64 the
# array is only ~70% busy at the 37.6 ns/group DMA pace and HAM demotes to
# 4/8 mid-phase (measured, costs ~2x); N=80 gives ~94% duty, right AT the
# threshold, and HAM flip-flops every 16384-cycle evaluation window
# (measured).  N=112 clears ~95.5% duty (the plain-S^T S baseline held at
# 95.2%) while still running each group ~7 ns faster than plain N=128.
NMOV = 112

_CACHE = {}
LAST_RESULT = None  # BassKernelResults of the most recent run (for test harness)


def _f8_dtype():
    import ml_dtypes

    return ml_dtypes.float8_e4m3


def _ensure_ntff_hook():
    """Install antenv.axon_hooks shim if missing, so run_bass_kernel_spmd
    trace=True can capture NTFF profiles via libaxon_pjrt.so ctypes calls.
    Only used when tracing is requested (test harness)."""
    import sys
    try:
        from antenv.axon_hooks import get_axon_ntff_profile_hook  # noqa: F401
        return
    except ImportError:
        pass
    import ctypes
    import contextlib
    import types

    so_path = "/opt/axon/libaxon_pjrt.so"
    hook = None
    if os.path.exists(so_path):
        lib = ctypes.CDLL(so_path)
        if hasattr(lib, "axon_start_nrt_profile"):
            lib.axon_start_nrt_profile.argtypes = [
                ctypes.POINTER(ctypes.c_int64),
                ctypes.c_size_t,
            ]
            lib.axon_start_nrt_profile.restype = ctypes.c_int64
            lib.axon_stop_nrt_profile.argtypes = [ctypes.c_char_p]
            lib.axon_stop_nrt_profile.restype = ctypes.c_int64

            @contextlib.contextmanager
            def _hook(output_dir, device_ids):
                import jax

                jax.devices()
                if device_ids:
                    ids = (ctypes.c_int64 * len(device_ids))(*device_ids)
                    rc = lib.axon_start_nrt_profile(ids, len(device_ids))
                else:
                    rc = lib.axon_start_nrt_profile(None, 0)
                if rc != 0:
                    raise RuntimeError(f"axon_start_nrt_profile rc={rc}")
                try:
                    yield
                finally:
                    n = lib.axon_stop_nrt_profile(str(output_dir).encode())
                    if n < 0:
                        raise RuntimeError(f"axon_stop_nrt_profile rc={n}")
                    print(f"profile: {n} file(s) written to {output_dir}")

            hook = _hook

    mod = types.ModuleType("antenv.axon_hooks")
    mod._hook = hook
    mod.get_axon_ntff_profile_hook = lambda: mod._hook
    mod.set_axon_ntff_profile_hook = lambda h: setattr(mod, "_hook", h)
    import antenv

    antenv.axon_hooks = mod
    sys.modules["antenv.axon_hooks"] = mod


def _build():
    """Build the per-core Bass program (SPMD, identical on all cores).

    Raw Bass with manual semaphores:
      sync/scalar : input dma_starts (queued back-to-back, one ring each),
                    sync also stores the PSUM gram to DRAM at the end
      tensor      : per segment wait for its DMA, then run its DoubleRow
                    LDW+MM pairs, all accumulating into one PSUM bank
    """
    import concourse.bass as bass
    import concourse.mybir as mybir

    nc = bass.Bass(
        "TRN2",
        target_bir_lowering=False,
        debug=False,
        enable_asserts=False,
        num_devices=NCORES,
        enable_partition_id=False,
    )
    x = nc.dram_tensor(
        "x", [128, GROUPS, 128], mybir.dt.float8e4, kind="ExternalInput"
    )
    out = nc.dram_tensor("out", [128, 64], mybir.dt.float32, kind="ExternalOutput")

    import contextlib

    with contextlib.ExitStack() as ctx:
        xsb = ctx.enter_context(
            nc.sbuf_tensor([128, GROUPS, 128], mybir.dt.float8e4)
        )
        osb = ctx.enter_context(nc.sbuf_tensor([128, 64], mybir.dt.float32))
        wsb = ctx.enter_context(nc.sbuf_tensor([128, 128], mybir.dt.float8e4))
        # Two full PSUM banks.  Bank 0 (free cols 0:512) holds the
        # even-chunk gram accumulator at partitions 0-63 (its cols 64:80 are
        # the NMOV cross-chunk garbage tail); bank 1 (free cols 512:1024)
        # holds the odd-chunk gram at partitions 64-127 (useful block at the
        # END of its 80-col window).  Disjoint zero regions AND disjoint PE
        # column groups -> the two matmuls of a group stream concurrently
        # through separate XBUSes.
        ps = ctx.enter_context(nc.psum_tensor([128, 1024], mybir.dt.float32))
        wps = ctx.enter_context(nc.psum_tensor([128, 128], mybir.dt.float32))
        seg_sems = [
            ctx.enter_context(nc.semaphore(name=f"seg_sem{s}")) for s in range(NSEG)
        ]
        warm_sem = ctx.enter_context(nc.semaphore(name="warm_sem"))
        mm_done = ctx.enter_context(nc.semaphore(name="mm_done"))
        copy_done = ctx.enter_context(nc.semaphore(name="copy_done"))
        out_sem = ctx.enter_context(nc.semaphore(name="out_sem"))
        block = ctx.enter_context(nc.Block())

        seg_start = [sum(SEG_GROUPS[:s]) for s in range(NSEG)]

        def issue_loads(eng, segs):
            for s in segs:
                g0, gn = seg_start[s], SEG_GROUPS[s]
                eng.dma_start(
                    out=xsb[:, g0 : g0 + gn], in_=x[:, g0 : g0 + gn]
                ).then_inc(seg_sems[s], 16)

        ring_segs = {
            e: [s for s in range(NSEG) if RING_ENGINES[s % len(RING_ENGINES)] == e]
            for e in RING_ENGINES
        }

        @block.sync
        def _(sync):
            issue_loads(sync, ring_segs.get("sync", []))
            sync.wait_ge(copy_done, 1)
            sync.dma_start(out=out[:], in_=osb[:]).then_inc(out_sem, 16)
            sync.wait_ge(out_sem, 16)

        @block.scalar
        def _(scalar):
            issue_loads(scalar, ring_segs.get("scalar", []))

        @block.vector
        def _(vector):
            vector.wait_ge(mm_done, 1)
            vector.tensor_copy(osb[0:64, :], ps[0:64, 0:64])
            vector.tensor_copy(
                osb[64:128, :], ps[64:128, 512 + NMOV - 64 : 512 + NMOV]
            ).then_inc(copy_done, 1)

        @block.gpsimd
        def _(gpsimd):
            gpsimd.memset(wsb[:], 0.0).then_inc(warm_sem, 1)

        @block.tensor
        def _(tensor):
            tensor.wait_ge(warm_sem, 1)
            for _ in range(WARMUP_MMS):
                tensor.matmul(wps[:], wsb[:], wsb[:], start=True, stop=True)
            # Every group: two concurrent col-tiled gram matmuls.  The
            # even chunk's stationary occupies PE columns 0-63 (psum
            # partitions 0-63), the odd chunk's PE columns 64-127 (psum
            # partitions 64-127), each streaming an NMOV-column window of
            # the group's 128 columns: [0:NMOV] for even (gram in out cols
            # 0:64), [128-NMOV:128] for odd (gram in the last 64 out cols).
            g = 0
            for s in range(NSEG):
                tensor.wait_ge(seg_sems[s], 16)
                for j in range(SEG_GROUPS[s]):
                    gg = seg_start[s] + j
                    te = xsb[:, gg, 0:64]
                    to = xsb[:, gg, 64:128]
                    me = xsb[:, gg, 0:NMOV]
                    mo = xsb[:, gg, 128 - NMOV : 128]
                    tensor.matmul(
                        ps[0:64, 0:NMOV], te, me,
                        start=(g == 0), stop=(g == GROUPS - 1),
                    )
                    mm = tensor.matmul(
                        ps[64:128, 512 : 512 + NMOV], to, mo,
                        start=(g == 0), stop=(g == GROUPS - 1),
                    )
                    g += 1
            mm.then_inc(mm_done, 1)

    return nc


def _prepare_inputs(pred, hr):
    """Pack p/h into the per-core transposed+interleaved fp8 layout.

    X[core][q, c, t, j] = (p if t==0 else h)[j, core*KC + c*128 + q]
    flattened to [128, GROUPS, 2, 64] per core (c = 2*g + parity, with the
    two chunks of group g side by side in the last-two axes as
    [p|h]_even, [p|h]_odd -> [128, g, (even|odd), (p32|h32)]).
    """
    f8 = _f8_dtype()
    p = np.asarray(pred).reshape(B, K).astype(f8)
    h = np.asarray(hr).reshape(B, K).astype(f8)
    p4 = p.reshape(B, NCORES, NCH, 128)
    h4 = h.reshape(B, NCORES, NCH, 128)
    xall = np.empty((NCORES, 128, NCH, 2, B), dtype=f8)
    xall[:, :, :, 0, :] = p4.transpose(1, 3, 2, 0)
    xall[:, :, :, 1, :] = h4.transpose(1, 3, 2, 0)
    return xall.reshape(NCORES, 128, GROUPS, 128)


def _finalize(R):
    """R: [128,64] float64 sum of per-core accumulated gram matrices:
    partitions 0..63 hold the even-chunk gram, 64..127 the odd-chunk gram
    (the two col-tiled PE halves).  Rows/cols 0..31 = pred, 32..63 = hr."""
    R = R[0:64] + R[64:128]
    Gpp = R[0:32, 0:32]
    Gph = R[0:32, 32:64]
    Ghh = R[32:64, 32:64]

    pn = np.sqrt(np.diag(Gpp))
    hn = np.sqrt(np.diag(Ghh))
    S_srhr = Gph / (pn[:, None] * hn[None, :])
    S_srsr = Gpp / (pn[:, None] * pn[None, :])
    hsq = np.diag(Ghh)
    d2 = np.maximum(hsq[:, None] + hsq[None, :] - 2.0 * Ghh, 0.0)
    dist = np.sqrt(d2)
    with np.errstate(divide="ignore"):
        M = np.minimum(-20.0 * np.log10(dist), 0.0)
    mask_pos = np.abs(M) > 30.0
    w = (np.exp(S_srsr) + 2.0 * np.exp(S_srhr)) / 0.5
    Qpos = np.where(mask_pos, w, 0.0).sum(axis=1)
    Qneg = np.where(mask_pos, 0.0, w).sum(axis=1)
    loss = (-1.0 / B) * np.sum(np.log(Qpos / Qneg))
    return np.asarray(loss, dtype=np.float32)


def kernel(pred, hr):
    global LAST_RESULT
    from concourse.bass_utils import run_bass_kernel_spmd

    trace = bool(os.environ.get("KERNEL_TRACE"))
    if trace:
        _ensure_ntff_hook()

    if "nc" not in _CACHE:
        _CACHE["nc"] = _build()
    nc = _CACHE["nc"]

    xall = _prepare_inputs(pred, hr)
    in_maps = [{"x": xall[c]} for c in range(NCORES)]
    # The axon-tunneled NeuronCores occasionally report a transient
    # unrecoverable-exec-unit error; recovery can take tens of seconds,
    # so back off with escalating sleeps before resubmitting.
    last_err = None
    res = None
    for attempt, backoff in enumerate([10.0, 30.0, 90.0, 0.0]):
        try:
            res = run_bass_kernel_spmd(
                nc, in_maps, core_ids=list(range(NCORES)), trace=trace and attempt == 0
            )
            break
        except Exception as e:  # noqa: BLE001
            last_err = e
            if backoff == 0.0:
                raise
            import time

            time.sleep(backoff)
    if res is None:
        raise last_err
    LAST_RESULT = res
    R = np.zeros((128, 64), dtype=np.float64)
    for c in range(NCORES):
        R += res.results[c]["out"].astype(np.float64)
    return _finalize(R)


# revision 26
# speedup vs baseline: 1.9301x; 1.1617x over previous
"""Trainium2 Bass kernel for nn_DCELoss (decoupled contrastive-style loss).

The whole loss reduces to three 32x32 gram matrices over the flattened
feature axis K = 96^3 = 884736:
    G_pp = p @ p.T,  G_ph = p @ h.T,  G_hh = h @ h.T
(row norms are their diagonals).  The final masked reduction is tiny 32x32
math done on host in float64.

Sharding: data-parallel over K across the 8 NeuronCores.  Each core gets a
K/8 slice, pre-packed on host into a transposed + interleaved fp8 layout
X[128, 432, 128]: group g holds two 128-k chunks side by side, each as 64
columns [p_rows(32) | h_rows(32)].  On device, each 128-column group is fed
to the PE array as BOTH the stationary and moving operand:
out[128,128] = S^T S accumulated in PSUM over all 432 groups; the host sums
the two diagonal 64x64 blocks (even/odd chunk grams) over cores.

Why this shape and not something cleverer (all measured on HW):
  * fp8 runs the PE at bf16 speed (1 moving col/cycle); the 2x DoubleRow
    mode disables Fast Weight Load, so for our FD=64 grams LDWEIGHTS
    dominates and it is a net LOSS (73 ns/group vs 56).
  * 2x col-tiling (even gram in PE cols 0-63, odd in 64-127, two
    concurrent N=64-112 matmuls) does reach ~27-50 ns/group warm, BUT
    (a) at <95% array duty the HAM clock-gate demotes 4/8 <-> 8/8 every
    16384-cycle window, and (b) 4 instructions/group trips the engine's
    16 KiB instruction-page demand-fetch (~1-3.4 us per page, queued
    behind input DMA), stalling ~10 us/run.  Plain S^T S (2 instr/group,
    95.2% duty) is the fastest structure that satisfies both walls.
  * The ~5-6 us HAM 1.2->2.4 GHz ramp is bridged with a short dummy-matmul
    burst only until the first DMA segment lands; the remaining ramp is
    absorbed by real (cold, ~107 ns) data matmuls, so ramp time does
    useful work instead of idling behind a fixed-length warmup.

fp8_e4m3 quantization of the inputs perturbs the final loss by ~3e-6
relative: the loss is a log of large masked sums of exp(cosine) terms with
cosines ~1e-3 over K ~ 1e6 elements, so elementwise rounding noise cancels
almost entirely.

Raw Bass (no Tile framework), engine bodies WITHOUT a Block end-barrier:
the NEFF postamble emitted by the compiler already ends with an all-engine
barrier + semaphore-file reset, so the Tile/Block gather-release chain
(~2 us across 5 engines) is pure overhead.
"""

import os
import numpy as np

B = 32
K = 884736
NCORES = 8
KC = K // NCORES            # 110592 k-values per core
NCH = KC // 128             # 864 chunks of 128 k-values
GROUPS = NCH // 2           # 432 matmul groups (2 chunks x 64 cols each)
# Input DMA segments, in units of 16 KiB groups (total 432).  Small head
# segments so the first matmuls start as soon as the ring delivers (~2.5 us
# ring startup latency); large middle segments give 4-8 KiB per-partition
# DMA lines (full SDMA rate); small tail segments so the last matmuls
# finish right behind the last DMA byte.  Segments alternate between the
# two HWDGE rings (sync / scalar engines).
SEG_GROUPS = [2, 6, 12, 24, 40, 56, 64, 64, 56, 44, 32, 20, 8, 4]
assert sum(SEG_GROUPS) == GROUPS
NSEG = len(SEG_GROUPS)
# Dense dummy matmuls bridge PE activity from engine start (~7.7 us) until
# the first data segment lands (~9.5-12 us); HAM needs ~5-6 us of sustained
# near-100% duty to lift the PE clock 1.2 -> 2.4 GHz, and the cold data
# matmuls after the bridge keep accumulating that credit on useful work.
WARMUP_MMS = 40

_CACHE = {}
LAST_RESULT = None  # BassKernelResults of the most recent run (for test harness)


def _f8_dtype():
    import ml_dtypes

    return ml_dtypes.float8_e4m3


def _ensure_ntff_hook():
    """Install antenv.axon_hooks shim if missing, so run_bass_kernel_spmd
    trace=True can capture NTFF profiles via libaxon_pjrt.so ctypes calls.
    Only used when tracing is requested (test harness)."""
    import sys
    try:
        from antenv.axon_hooks import get_axon_ntff_profile_hook  # noqa: F401
        return
    except ImportError:
        pass
    import ctypes
    import contextlib
    import types

    so_path = "/opt/axon/libaxon_pjrt.so"
    hook = None
    if os.path.exists(so_path):
        lib = ctypes.CDLL(so_path)
        if hasattr(lib, "axon_start_nrt_profile"):
            lib.axon_start_nrt_profile.argtypes = [
                ctypes.POINTER(ctypes.c_int64),
                ctypes.c_size_t,
            ]
            lib.axon_start_nrt_profile.restype = ctypes.c_int64
            lib.axon_stop_nrt_profile.argtypes = [ctypes.c_char_p]
            lib.axon_stop_nrt_profile.restype = ctypes.c_int64

            @contextlib.contextmanager
            def _hook(output_dir, device_ids):
                import jax

                jax.devices()
                if device_ids:
                    ids = (ctypes.c_int64 * len(device_ids))(*device_ids)
                    rc = lib.axon_start_nrt_profile(ids, len(device_ids))
                else:
                    rc = lib.axon_start_nrt_profile(None, 0)
                if rc != 0:
                    raise RuntimeError(f"axon_start_nrt_profile rc={rc}")
                try:
                    yield
                finally:
                    n = lib.axon_stop_nrt_profile(str(output_dir).encode())
                    if n < 0:
                        raise RuntimeError(f"axon_stop_nrt_profile rc={n}")
                    print(f"profile: {n} file(s) written to {output_dir}")

            hook = _hook

    mod = types.ModuleType("antenv.axon_hooks")
    mod._hook = hook
    mod.get_axon_ntff_profile_hook = lambda: mod._hook
    mod.set_axon_ntff_profile_hook = lambda h: setattr(mod, "_hook", h)
    import antenv

    antenv.axon_hooks = mod
    sys.modules["antenv.axon_hooks"] = mod


def _build():
    """Build the per-core Bass program (SPMD, identical on all cores).

    Raw Bass with manual semaphores and hand-rolled engine bodies (no Block
    end-barrier):
      sync/scalar : input dma_starts (queued back-to-back, one ring each),
                    sync also does the output store at the end
      tensor      : HAM-bridge dummy matmuls, then per segment wait for its
                    DMA and run its LDW+MM pairs, all accumulating into one
                    PSUM bank
      vector      : single PSUM -> SBUF copy after the last matmul
      gpsimd      : memset of the dummy-matmul scratch tile
    """
    import concourse.bass as bass
    import concourse.mybir as mybir

    nc = bass.Bass(
        "TRN2",
        target_bir_lowering=False,
        debug=False,
        enable_asserts=False,
        num_devices=NCORES,
        enable_partition_id=False,
    )
    x = nc.dram_tensor(
        "x", [128, GROUPS, 128], mybir.dt.float8e4, kind="ExternalInput"
    )
    out = nc.dram_tensor("out", [128, 128], mybir.dt.float32, kind="ExternalOutput")

    import contextlib

    with contextlib.ExitStack() as ctx:
        xsb = ctx.enter_context(
            nc.sbuf_tensor([128, GROUPS, 128], mybir.dt.float8e4)
        )
        osb = ctx.enter_context(nc.sbuf_tensor([128, 128], mybir.dt.float32))
        wsb = ctx.enter_context(nc.sbuf_tensor([128, 128], mybir.dt.float8e4))
        ps = ctx.enter_context(nc.psum_tensor([128, 128], mybir.dt.float32))
        wps = ctx.enter_context(nc.psum_tensor([128, 128], mybir.dt.float32))
        seg_sems = [
            ctx.enter_context(nc.semaphore(name=f"seg_sem{s}")) for s in range(NSEG)
        ]
        warm_sem = ctx.enter_context(nc.semaphore(name="warm_sem"))
        mm_done = ctx.enter_context(nc.semaphore(name="mm_done"))
        copy_done = ctx.enter_context(nc.semaphore(name="copy_done"))
        out_sem = ctx.enter_context(nc.semaphore(name="out_sem"))

        seg_start = [sum(SEG_GROUPS[:s]) for s in range(NSEG)]

        def issue_loads(eng, segs):
            for s in segs:
                g0, gn = seg_start[s], SEG_GROUPS[s]
                eng.dma_start(
                    out=xsb[:, g0 : g0 + gn], in_=x[:, g0 : g0 + gn]
                ).then_inc(seg_sems[s], 16)

        def body_sync(sync):
            issue_loads(sync, range(0, NSEG, 2))
            sync.wait_ge(copy_done, 1)
            sync.dma_start(out=out[:], in_=osb[:]).then_inc(out_sem, 16)
            sync.wait_ge(out_sem, 16)

        def body_scalar(scalar):
            issue_loads(scalar, range(1, NSEG, 2))

        def body_vector(vector):
            vector.wait_ge(mm_done, 1)
            vector.tensor_copy(osb[:], ps[:]).then_inc(copy_done, 1)

        def body_gpsimd(gpsimd):
            gpsimd.memset(wsb[:], 0.0).then_inc(warm_sem, 1)

        def body_tensor(tensor):
            tensor.wait_ge(warm_sem, 1)
            for _ in range(WARMUP_MMS):
                tensor.matmul(wps[:], wsb[:], wsb[:], start=True, stop=True)
            g = 0
            for s in range(NSEG):
                tensor.wait_ge(seg_sems[s], 16)
                for j in range(SEG_GROUPS[s]):
                    sl = xsb[:, seg_start[s] + j]
                    mm = tensor.matmul(
                        ps[:], sl, sl, start=(g == 0), stop=(g == GROUPS - 1)
                    )
                    g += 1
            mm.then_inc(mm_done, 1)

        # Hand-rolled engine bodies: same per-engine basic-block structure a
        # Bass Block() emits, minus its end-of-block all-engine barrier
        # (drain + gather/release EVSEM chain, ~2 us across 5 engines).  The
        # compiler-emitted NEFF postamble that follows already begins with
        # its own all-engine barrier, and the out_sem wait keeps the output
        # DMA inside the kernel body.
        end_bb = "prog_end"
        for eng, fn in (
            (nc.sync, body_sync),
            (nc.scalar, body_scalar),
            (nc.vector, body_vector),
            (nc.gpsimd, body_gpsimd),
            (nc.tensor, body_tensor),
        ):
            bb = f"body_{eng.engine.value}"
            eng.br(bb)
            with nc.body(bb):
                fn(eng)
                eng.br(end_bb)
        nc.switch_bb(end_bb)

    return nc


def _prepare_inputs(pred, hr):
    """Pack p/h into the per-core transposed+interleaved fp8 layout.

    X[core][q, c, t, j] = (p if t==0 else h)[j, core*KC + c*128 + q]
    flattened to [128, GROUPS, 128] per core: group g's 128 columns are
    [p|h of chunk 2g (64) | p|h of chunk 2g+1 (64)].
    """
    f8 = _f8_dtype()
    p = np.asarray(pred).reshape(B, K).astype(f8)
    h = np.asarray(hr).reshape(B, K).astype(f8)
    p4 = p.reshape(B, NCORES, NCH, 128)
    h4 = h.reshape(B, NCORES, NCH, 128)
    xall = np.empty((NCORES, 128, NCH, 2, B), dtype=f8)
    xall[:, :, :, 0, :] = p4.transpose(1, 3, 2, 0)
    xall[:, :, :, 1, :] = h4.transpose(1, 3, 2, 0)
    return xall.reshape(NCORES, 128, GROUPS, 128)


def _finalize(R):
    """R: [128,128] float64 sum of per-core accumulated S^T S matrices.
    Diagonal 64x64 blocks are the even/odd chunk grams; within a block,
    rows/cols 0..31 = pred rows, 32..63 = hr rows."""
    R = R[0:64, 0:64] + R[64:128, 64:128]
    Gpp = R[0:32, 0:32]
    Gph = R[0:32, 32:64]
    Ghh = R[32:64, 32:64]

    pn = np.sqrt(np.diag(Gpp))
    hn = np.sqrt(np.diag(Ghh))
    S_srhr = Gph / (pn[:, None] * hn[None, :])
    S_srsr = Gpp / (pn[:, None] * pn[None, :])
    hsq = np.diag(Ghh)
    d2 = np.maximum(hsq[:, None] + hsq[None, :] - 2.0 * Ghh, 0.0)
    dist = np.sqrt(d2)
    with np.errstate(divide="ignore"):
        M = np.minimum(-20.0 * np.log10(dist), 0.0)
    mask_pos = np.abs(M) > 30.0
    w = (np.exp(S_srsr) + 2.0 * np.exp(S_srhr)) / 0.5
    Qpos = np.where(mask_pos, w, 0.0).sum(axis=1)
    Qneg = np.where(mask_pos, 0.0, w).sum(axis=1)
    loss = (-1.0 / B) * np.sum(np.log(Qpos / Qneg))
    return np.asarray(loss, dtype=np.float32)


def kernel(pred, hr):
    global LAST_RESULT
    from concourse.bass_utils import run_bass_kernel_spmd

    trace = bool(os.environ.get("KERNEL_TRACE"))
    if trace:
        _ensure_ntff_hook()

    if "nc" not in _CACHE:
        _CACHE["nc"] = _build()
    nc = _CACHE["nc"]

    xall = _prepare_inputs(pred, hr)
    in_maps = [{"x": xall[c]} for c in range(NCORES)]
    # The axon-tunneled NeuronCores occasionally report a transient
    # unrecoverable-exec-unit error; recovery can take tens of seconds,
    # so back off with escalating sleeps before resubmitting.
    last_err = None
    res = None
    for attempt, backoff in enumerate([10.0, 30.0, 90.0, 0.0]):
        try:
            res = run_bass_kernel_spmd(
                nc, in_maps, core_ids=list(range(NCORES)), trace=trace and attempt == 0
            )
            break
        except Exception as e:  # noqa: BLE001
            last_err = e
            if backoff == 0.0:
                raise
            import time

            time.sleep(backoff)
    if res is None:
        raise last_err
    LAST_RESULT = res
    R = np.zeros((128, 128), dtype=np.float64)
    for c in range(NCORES):
        R += res.results[c]["out"].astype(np.float64)
    return _finalize(R)
